# revision 11
# baseline (speedup 1.0000x reference)
"""Trainium2 Bass kernel for the quirky-reshape MultiHeadSelfAttention layer.

Reference math (B=1, S=2048, H=768):
    qkv = x @ W_qkv                  # (S, 2304)
    col c' = h*36 + t ; h in [0,64) "heads", t in [0,36): q=t<12, k=12<=t<24, v=t>=24
    per head h (d_k=12):  A_h = softmax(Q_h K_h^T / sqrt(12));  O_h = A_h V_h
    values[:, h*12+d] = O_h[:, d];   out = values @ W_o
Sharding: 8 heads per core (tensor-parallel over the 64-head axis); host
sums the 8 rank-96 partial output projections.

The exp stream is the hard wall: 33.5M logits/core through ACT (exact
Exp, 153.6 G elem/s marginal) + DVE (fast-exp int16 bit trick, 122.9
G elem/s, PSUM-source fp32 = 1x mode).  Everything else is scheduled
into that shadow:
  * QKV projection is packed 4-heads-per-128-col weight tile (2 tiles
    per t instead of 6 group tiles), and emitted just-in-time: a short
    ramp computes K/Q for chunk 0 only, then the remaining K/Q/V chains
    are injected in PAIRS between j-blocks of chunk 0's attention,
    each placed just before the j-block that needs it.
  * Attention per (i-chunk 512, j-block 128, 4 head-groups of 2):
    transposed logits A^T[j,i] via row-tiled fp16 matmuls into 2-bank
    PSUM tiles on a 3-deep rotation (deeper rotation = shorter
    round->exp->reuse cadence); exp split ~4.5 ACT / 3.5 DVE per
    j-block; attention@V as one 8-matmul batch per j-block sorted by
    PSUM bank, riding mid-jb where it fills exp-wait PE idle.
  * Output is fp16 (the host sum of 8 rank-96 partials tolerates it);
    halves the 6 MB output DMA in the tail.
  * AV drains are 2 full-width copies (engine cost is free-dim-based,
    so [13,512] costs the same as [128,512]); the av PSUM banks are
    memset once in the ramp so never-written rows stay finite (later
    chunks inherit finite rows from the bc sel-matmul).
  * Normalization + output projection as in the baseline: selector-
    matmul denominator broadcast, fast reciprocal, multiply; PO as a
    dense tail stream with 3-deep PSUM rotation and zero-padded W_o.
"""

import numpy as np

import concourse.bass as bass
import concourse.mybir as mybir
import concourse.tile as tile
from concourse import bacc
from concourse.bass_utils import run_bass_kernel_spmd

F32 = mybir.dt.float32
F32R = mybir.dt.float32r
FP16 = mybir.dt.float16
I16 = mybir.dt.int16

S = 2048
H = 768
DK = 12            # per-head dim (reference N_HEADS)
HEADS_PER_CORE = 8
N_CORES = 8
SCALE = 1.0 / float(np.sqrt(DK))

# DVE fast-exp: write fp16 BIT PATTERNS via int16 tensor_scalar.
#   bits = round(L * C1 + C2)  ->  bitcast fp16 ~= exp(SCALE * L)
FE_C1 = float(1024.0 * np.log2(np.e) * SCALE)
FE_C2 = float(15360.0 - 58.7)
# logits head groups: 4 groups x 2 slots on a 3-deep rotation of 2-bank
# PSUM tiles.  The steady-state cadence of a d-deep rotation is
# (groups/jb / d) * [round + sem + exp + WAR] -- the old 2-deep/3-group
# layout measured 4.3us/jb; 3-deep/4-group models ~3.3.  Head h maps to
# PE row-tile 32*(h%4), so rounds {h0..h3},{h4..h7} run 4-wide.
GROUPS = ((0, 2), (1, 2), (2, 2), (3, 2))
_G_BASE = (0, 2, 4, 6)


def _head_of(g, s):
    return _G_BASE[g] + s


def _exp_on_act(g, jb):
    # ACT: g0, g2 always + g3 every 4th jb; DVE: g1 + g3 otherwise.
    # Measured: ACT 73.7% vs DVE 54.7% busy at a 1:1 g3 alternation, and
    # a 2-slot DVE fast-exp (1200ns) costs about the same as ACT (1147).
    return g in (0, 2) or (g == 3 and jb % 4 == 3)


def build_program():
    nc = bacc.Bacc("TRN2", target_bir_lowering=False, debug=False)

    xt_d = nc.dram_tensor("xt", [H, S], FP16, kind="ExternalInput")
    wqk_d = nc.dram_tensor("wqk", [H, 2, 2, 128], FP16, kind="ExternalInput")
    wv_d = nc.dram_tensor("wv", [H, 96], FP16, kind="ExternalInput")
    wo_d = nc.dram_tensor("wo", [2, 128, H], F32R, kind="ExternalInput")
    sel_d = nc.dram_tensor("sel", [128, 128], F32R, kind="ExternalInput")
    out_d = nc.dram_tensor("out", [S, H], FP16, kind="ExternalOutput")

    with tile.TileContext(nc) as tc:
        with tc.tile_pool(name="const", bufs=1) as cpool, \
             tc.tile_pool(name="ps_l", bufs=3, space="PSUM") as ps_l, \
             tc.tile_pool(name="ps_av", bufs=1, space="PSUM") as ps_av, \
             tc.tile_pool(name="esb", bufs=11) as esb, \
             tc.tile_pool(name="esb2", bufs=6) as esb2, \
             tc.tile_pool(name="osb", bufs=4) as opool:
            xt = cpool.tile([128, 6, S], FP16, tag="xt")
            wqk = cpool.tile([128, 6, 2, 2, 128], FP16, tag="wqk")
            wv = cpool.tile([128, 6, 96], FP16, tag="wv")
            wo = cpool.tile([128, 2, H], F32R, tag="wo")
            sel = cpool.tile([128, 128], F32R, tag="sel")
            qkt = cpool.tile([128, 2, 2, S], FP16, tag="qkt")
            vsb = cpool.tile([128, 16, 8, 13], FP16, tag="vsb")
            vhat = cpool.tile([128, 2, S], F32R, tag="vhat")

            # DMA order = need order: K weights, then x chunk 0 (the ramp
            # computes K/Q/V for chunk 0 only), then the rest streams in
            # under the early attention shadow.
            xt_r = xt_d.rearrange("(hb p) s -> p hb s", p=128)
            wqk_r = wqk_d.rearrange("(hb p) t g m -> p hb t g m", p=128)
            nc.sync.dma_start(wqk[:], wqk_r[:])
            nc.sync.dma_start(xt[:, :, 0:512], xt_r[:, :, 0:512])
            nc.sync.dma_start(wv[:], wv_d.rearrange("(hb p) n -> p hb n", p=128))
            for ch in (1, 2, 3):
                nc.sync.dma_start(xt[:, :, ch * 512:(ch + 1) * 512],
                                  xt_r[:, :, ch * 512:(ch + 1) * 512])
            nc.sync.dma_start(wo[:], wo_d.rearrange("b p o -> p b o"))
            nc.sync.dma_start(sel[:], sel_d[:])
            # ones column (index 12) for the denominator trick; V columns
            # 0..11 get overwritten by emit_v below.  GPSIMD: idle engine,
            # SBUF-only access is all this needs.
            nc.gpsimd.memset(vsb[:], 1.0)

            # ---- QKV producers (PSUM via shared L-slots) ----
            def emit_qk(t, tl, ch, drain):
                p = ps_l.tile([128, 2, 512], F32, tag="L",
                              name=f"pqk{t}{tl}{ch}")
                for hb in range(6):
                    nc.tensor.matmul(
                        p[:, 0, :],
                        lhsT=wqk[:, hb, t, tl, :],
                        rhs=xt[:, hb, ch * 512:(ch + 1) * 512],
                        start=(hb == 0),
                        stop=(hb == 5),
                    )
                if drain == 0:
                    nc.scalar.copy(qkt[:, t, tl, ch * 512:(ch + 1) * 512],
                                   p[:, 0, :])
                else:
                    nc.vector.tensor_copy(qkt[:, t, tl, ch * 512:(ch + 1) * 512],
                                          p[:, 0, :])

            def emit_v(sb):
                p = ps_l.tile([128, 2, 512], F32, tag="L", name=f"pv{sb}")
                for hb in range(6):
                    nc.tensor.matmul(
                        p[:, 0, :96],
                        lhsT=xt[:, hb, sb * 128:(sb + 1) * 128],
                        rhs=wv[:, hb, :],
                        start=(hb == 0),
                        stop=(hb == 5),
                    )
                nc.vector.tensor_copy(
                    vsb[:, sb, :, 0:12],
                    p[:, 0, :96].rearrange("p (h d) -> p h d", d=12),
                )

            # ---- per-chunk epilogue steps (normalize + project + store) ----
            def emit_bc(ic, b):
                # runs in the av slot of its quad, before the next chunk's
                # av accumulators are allocated; the sel matmul writes all
                # 128 partitions, which also re-finitizes the bank's
                # never-written rows for the next chunk's full-width drain
                lo, hi = ic * 512, (ic + 1) * 512
                bc = ps_av.tile([128, 512], F32, tag=f"av{b}", name=f"bc{b}_{ic}")
                nc.tensor.matmul(bc[:], lhsT=sel[:], rhs=vhat[:, b, lo:hi],
                                 start=True, stop=True)
                nc.vector.reciprocal_approx_fast(bc[:], bc[:])
                nc.vector.tensor_tensor(vhat[:, b, lo:hi], vhat[:, b, lo:hi],
                                        bc[:], mybir.AluOpType.mult)

            def po_mm(ib):
                # tail PO matmuls: emitted 3 ibs ahead of their drains so
                # the PE stream is dense (HAM stays warm) across a ~4-deep
                # PSUM rotation (three L-pool slots + the freed av banks)
                if ib % 3 == 2:
                    poa = ps_av.tile([128, 512], F32, tag="av0",
                                     name=f"poa_{ib}")
                    pob = ps_av.tile([128, 512], F32, tag="av1",
                                     name=f"pob_{ib}")
                    pa, pb = poa[:], pob[:, :256]
                else:
                    pt = ps_l.tile([128, 2, 512], F32, tag="L",
                                   name=f"po_{ib}")
                    pa, pb = pt[:, 0, :], pt[:, 1, :256]
                for b in range(2):
                    nc.tensor.matmul(
                        pa,
                        lhsT=vhat[:, b, ib * 128:(ib + 1) * 128],
                        rhs=wo[:, b, 0:512],
                        start=(b == 0), stop=(b == 1),
                    )
                for b in range(2):
                    nc.tensor.matmul(
                        pb,
                        lhsT=vhat[:, b, ib * 128:(ib + 1) * 128],
                        rhs=wo[:, b, 512:768],
                        start=(b == 0), stop=(b == 1),
                    )
                return pa, pb

            def po_drain(ib, pa, pb):
                # both engines idle at the tail: alternate which gets the
                # wide half so the two drain streams stay balanced
                osb = opool.tile([128, 768], FP16, tag="osb", name=f"osb_{ib}")
                if ib % 2 == 0:
                    nc.scalar.copy(osb[:, 0:512], pa)
                    nc.vector.tensor_copy(osb[:, 512:768], pb)
                else:
                    nc.vector.tensor_copy(osb[:, 0:512], pa)
                    nc.scalar.copy(osb[:, 512:768], pb)
                nc.sync.dma_start(out_d[ib * 128:(ib + 1) * 128, :], osb[:])

            def emit_av_jb(av, jb, tiles):
                # All 8 AV matmuls of one j-block, issued contiguously and
                # sorted by PSUM bank so each bank's four col-tiles run
                # concurrently: exactly 2 serial PE rounds per j-block.
                for b in range(2):
                    for c in range(4):
                        h = 4 * b + c
                        g, s = divmod(h, 2)
                        Ea, Eb, na = tiles[g]
                        rhs = Ea[:, s, :] if s < na else Eb[:, s, :]
                        # has_written tracking is per-partition, so the four
                        # col-slots of one bank are independent accumulation
                        # groups (disjoint partitions).
                        nc.tensor.matmul(
                            av[b][32 * c:32 * c + 13, :],
                            lhsT=vsb[:, jb, h, :],
                            rhs=rhs,
                            start=(jb == 0),
                            stop=(jb == 15),
                            tile_position=(0, 32 * c),
                            # CoreSim's group checker is not partition-aware;
                            # the pending-zero numerics are.
                            skip_group_check=True,
                        )

            def emit_vhat(ic, avp):
                # 2 full-width drains: per-partition engines price by free
                # dim, so 128 rows cost the same as 13; rows outside the
                # 4x13 data blocks are finite (ramp memset / bc leftovers)
                # and hit zero-padded W_o rows in the output projection.
                nc.scalar.copy(vhat[:, 0, ic * 512:(ic + 1) * 512], avp[0][:])
                nc.vector.tensor_copy(vhat[:, 1, ic * 512:(ic + 1) * 512],
                                      avp[1][:])

            # ---- injection schedules ----
            # QKV chains ride the attention shadow in PAIRS (even L-pool
            # allocation count keeps the ping-pong parity); each pair is
            # placed just before the j-block that first needs it.
            sched = {ic: {} for ic in range(4)}

            def add(ic, gidx, thunk):
                sched[ic].setdefault(max(0, gidx), []).append(thunk)

            for ic in (1, 2, 3):
                add(ic, 0, (lambda ic=ic: emit_bc(ic - 1, 0)))
                add(ic, 1, (lambda ic=ic: emit_bc(ic - 1, 1)))

            def add_pair(ic, jb, thunks):
                for th in thunks:
                    add(ic, 4 * jb + 3, th)

            def qk_pair(t, ch):
                return [lambda: emit_qk(t, 0, ch, 0),
                        lambda: emit_qk(t, 1, ch, 1)]

            def v_pair(sb):
                return [lambda: emit_v(sb), lambda: emit_v(sb + 1)]

            add_pair(0, 0, v_pair(0))
            add_pair(0, 1, v_pair(2))
            add_pair(0, 2, qk_pair(1, 1))     # K ch1: needed by jb4
            add_pair(0, 3, v_pair(4))
            add_pair(0, 4, v_pair(6))
            add_pair(0, 6, qk_pair(1, 2))     # K ch2: needed by jb8
            add_pair(0, 7, v_pair(8))
            add_pair(0, 8, v_pair(10))
            add_pair(0, 10, qk_pair(1, 3))    # K ch3: needed by jb12
            add_pair(0, 11, v_pair(12))
            add_pair(0, 12, v_pair(14))
            add_pair(0, 14, qk_pair(0, 1))    # Q ch1: needed by ic1
            add_pair(1, 1, qk_pair(0, 2))     # Q ch2: needed by ic2
            add_pair(2, 1, qk_pair(0, 3))     # Q ch3: needed by ic3

            # ---- ramp: chunk-0 K/Q, then attention ----
            emit_qk(1, 0, 0, 1)
            emit_qk(1, 1, 0, 0)
            emit_qk(0, 0, 0, 1)
            emit_qk(0, 1, 0, 0)

            # chunk 0's av accumulators: allocated in the ramp so their
            # never-written rows can be memset finite before the full-width
            # drain (later chunks are re-finitized by the bc sel-matmul).
            av0 = [ps_av.tile([128, 512], F32, tag=f"av{b}", name=f"av{b}_0")
                   for b in range(2)]
            nc.vector.memset(av0[0][:], 0.0)
            nc.vector.memset(av0[1][:], 0.0)

            # ---- attention: per-chunk loop with rolling AV lag ----
            LAG_JB = 3
            for ic in range(4):
                ready = []   # completed j-blocks: (jb, [(E, E2, na), ...])
                av = av0 if ic == 0 else None
                gidx = 0
                for jb in range(16):
                    cur = []
                    for g, nslots in GROUPS:
                        # the AV batch for the lagged j-block rides between
                        # g1's and g2's logits: it fills the PE while g2
                        # waits for exp(g0) to free the shared PSUM banks
                        if g == 2 and len(ready) > LAG_JB:
                            if av is None:
                                av = [ps_av.tile([128, 512], F32,
                                                 tag=f"av{b}",
                                                 name=f"av{b}_{ic}")
                                      for b in range(2)]
                            jbr, tiles = ready.pop(0)
                            emit_av_jb(av, jbr, tiles)
                        L = ps_l.tile([128, 2, 512], F32, tag="L",
                                      name=f"L_{ic}_{jb}_{g}")
                        for s in range(nslots):
                            h = _head_of(g, s)
                            tl, o = h // 4, 32 * (h % 4)
                            nc.tensor.matmul(
                                L[:, s, :],
                                lhsT=qkt[o:o + 12, 1, tl,
                                         jb * 128:(jb + 1) * 128],
                                rhs=qkt[o:o + 12, 0, tl,
                                        ic * 512:(ic + 1) * 512],
                                start=True,
                                stop=True,
                                tile_position=(o, 0),
                            )
                        # Exp per group, engine chosen by EXP_NA[g]: first
                        # `na` slots on ACT (exact exp), the rest on DVE
                        # (fast-exp bit trick).  Separate SBUF tiles per
                        # engine keep the writes independent (a shared tile
                        # serializes on the tile-granular WAW check).
                        na = nslots if _exp_on_act(g, jb) else 0
                        E = E2 = None
                        if na > 0:
                            E = esb.tile([128, 2, 512], FP16, tag="E",
                                         name=f"E_{ic}_{jb}_{g}")
                        if na < nslots:
                            E2 = esb2.tile([128, 2, 512], FP16, tag="E2",
                                           name=f"E2_{ic}_{jb}_{g}")
                        if na > 0:
                            nc.scalar.activation(
                                E[:, :na, :],
                                L[:, :na, :],
                                mybir.ActivationFunctionType.Exp,
                                scale=SCALE,
                            )
                        if na < nslots:
                            nc.vector.tensor_scalar(
                                E2[:, na:nslots, :].bitcast(I16),
                                L[:, na:nslots, :],
                                FE_C1, FE_C2,
                                mybir.AluOpType.mult, mybir.AluOpType.add,
                            )
                        for thunk in sched[ic].get(gidx, ()):
                            thunk()
                        cur.append((E, E2, na))
                        gidx += 1
                    ready.append((jb, cur))
                for jbr, tiles in ready:
                    emit_av_jb(av, jbr, tiles)
                emit_vhat(ic, av)
            # epilogue tail: chunks 0-2's output projections don't need the
            # last chunk's normalization, so they stream first (no bc3
            # latency before the PE warms); bc3 + the last 4 ride behind.
            pend = {}
            for ib in range(12):
                pend[ib] = po_mm(ib)
                if ib >= 3:
                    po_drain(ib - 3, *pend.pop(ib - 3))
            emit_bc(3, 0)
            emit_bc(3, 1)
            for ib in range(12, 16):
                pend[ib] = po_mm(ib)
                po_drain(ib - 3, *pend.pop(ib - 3))
            for ib in range(13, 16):
                po_drain(ib, *pend.pop(ib))

    nc.compile()
    return nc


def make_core_inputs(x, W_qkv, W_o):
    """Host-side shard/prepack. Returns list of per-core input dicts."""
    x = np.asarray(x, np.float32)
    W_qkv = np.asarray(W_qkv, np.float32)
    W_o = np.asarray(W_o, np.float32)
    xt = np.ascontiguousarray(x.reshape(S, H).T).astype(np.float16)  # [H, S]

    sel = np.zeros((128, 128), np.float32)
    for s4 in range(4):
        sel[32 * s4 + 12, 32 * s4:32 * (s4 + 1)] = 1.0

    in_maps = []
    for core in range(N_CORES):
        wqk = np.zeros((H, 2, 2, 128), np.float16)
        wv = np.zeros((H, 96), np.float16)
        wo = np.zeros((2, 128, H), np.float32)
        for h in range(HEADS_PER_CORE):
            Hg = HEADS_PER_CORE * core + h
            tl, o = divmod(h, 4)
            for t in range(2):
                wqk[:, t, tl, 32 * o:32 * o + 12] = \
                    W_qkv[:, Hg * 36 + t * 12:Hg * 36 + (t + 1) * 12]
            wv[:, 12 * h:12 * (h + 1)] = W_qkv[:, Hg * 36 + 24:Hg * 36 + 36]
            b, c = divmod(h, 4)
            wo[b, 32 * c:32 * c + 12, :] = W_o[Hg * DK:(Hg + 1) * DK, :]
        in_maps.append({"xt": xt, "wqk": wqk, "wv": wv, "wo": wo, "sel": sel})
    return in_maps


_NC_CACHE = None


def kernel(x, W_qkv, W_o):
    global _NC_CACHE
    if _NC_CACHE is None:
        _NC_CACHE = build_program()
    nc = _NC_CACHE
    in_maps = make_core_inputs(x, W_qkv, W_o)
    res = run_bass_kernel_spmd(nc, in_maps, core_ids=list(range(N_CORES)))
    out = np.zeros((S, H), np.float64)
    for r in res.results:
        out += r["out"].astype(np.float64)
    return out.astype(np.float32).reshape(1, S, H)
